# revision 39
# baseline (speedup 1.0000x reference)
"""MultiHeadAttention (B=4, S=2048, D=1024, H=16, rel-pos bias) on 8 TRN2 cores.

Sharding: core c -> batch b=c//2, head-group g=c%2 (8 heads each).
Per-core kernel computes partial out^T = Wo_g @ ctx_g^T  [1024, 2048] fp32;
host sums the two head-group partials per batch, transposes, adds bo.

Key-compaction: the mask zeroes ~half the keys; the host gathers only
unmasked keys (padded to NKPAD, a multiple of 128, uniform across cores)
for K/V, so QK, softmax, and PV shrink ~2x. The rel-pos bias is then no
longer Toeplitz in compacted key space, so the host builds exact per-
(key-tile, query-half) bias tiles in fp8 (values are tiny; quantization
is ~1e-3 of the exp argument), added via identity-matmul into PSUM.
Tiles where the clamped bias is constant (far from the diagonal for all
cores' masks) fold the constant into the per-partition exp bias instead.
The program is compiled per mask content (cached on mask bytes).

Per-core layouts:
  x^T      host-pre-transposed [8, 128, S|NKPAD] fp16 chunks
  Q^T/K^T  [128, 4, S|NKPAD]   partition = head-pair feature (2 heads x 64)
  V_aug    [128, NT, 520]  compacted keys on partitions; per head [V_h | 1]
  S^T      PSUM [128, 1024] keys x queries; exp on ACT (scale=0.125,
           per-partition bias = padding mask (+ const rel bias if folded))
  ctx^T    PSUM [65, 1024]  row 64 = softmax denominator ([V|1] trick);
           normalize: DVE copies den/ctx to SBUF (frees PSUM fast) ->
           reciprocal_approx_fast -> gpsimd partition_broadcast -> DVE mul.
Warm-up and boundary filler matmuls keep the PE HAM clock at 8/8; output
is written f16 (host sums the two head-group partials in f32).
"""

import hashlib
import numpy as np
import ml_dtypes

S = 2048
D = 1024
B = 4
NCORES = 8
HPC = 8   # heads per core
NPAIR = 4
MASK_NEG = -30000.0

_CACHE = {}


def _plan(mask):
    """Mask-dependent compaction plan, uniform across cores."""
    idxs = []
    for b in range(B):
        m = np.asarray(mask[b, 0, 0, :]).astype(np.int64)
        idxs.append(np.where(m != 0)[0])
    nk = max(len(ix) for ix in idxs)
    NT = max((nk + 127) // 128, 1)
    NKPAD = NT * 128
    # per (i, jh) classification intersected over cores (each core = one batch's mask)
    skips = {}
    for i in range(NT):
        for jh in range(2):
            qlo, qhi = jh * 1024, jh * 1024 + 1023
            okL = okR = True
            for ix in idxs:
                ko = ix[128 * i:128 * (i + 1)]
                if len(ko) == 0:
                    continue  # all-padding tile: compatible with either side
                okL = okL and (ko.max() <= qlo - 128)
                okR = okR and (ko.min() >= qhi + 128)
            if okR:
                skips[(i, jh)] = 'R'
            elif okL:
                skips[(i, jh)] = 'L'
    chunks = {jh: [i for i in range(NT) if (i, jh) not in skips]
              for jh in range(2)}
    cmax = max(1, max(len(chunks[0]), len(chunks[1])))
    # K-projection sequence blocks over NKPAD
    kblocks = []
    off = 0
    while off < NKPAD:
        w = min(1024, NKPAD - off)
        kblocks.append((off, w))
        off += w
    return dict(idxs=idxs, NT=NT, NKPAD=NKPAD, skips=skips, chunks=chunks,
                cmax=cmax, kblocks=kblocks)


def _build(plan):
    import concourse.bass as bass
    import concourse.mybir as mybir
    from concourse import bacc, tile
    from concourse.masks import make_identity

    f16 = mybir.dt.float16
    f8 = mybir.dt.float8e4
    f32 = mybir.dt.float32
    AF = mybir.ActivationFunctionType

    NT, NKPAD = plan['NT'], plan['NKPAD']
    skips, chunks, cmax = plan['skips'], plan['chunks'], plan['cmax']
    ci = {jh: {i: n for n, i in enumerate(chunks[jh])} for jh in range(2)}

    nc = bacc.Bacc("TRN2", target_bir_lowering=False, debug=False,
                   num_devices=NCORES)

    def din(name, shape, dt=f16):
        return nc.dram_tensor(name, shape, dt, kind="ExternalInput").ap()

    xq_d = din("xqt", [8, 128, 2048])
    xk_d = din("xkt", [8, 128, NKPAD])
    xv_d = din("xvt", [8, 128, NKPAD])
    wq_d = din("wq", [128, 8, 512])
    wk_d = din("wk", [128, 8, 512])
    wv_d = din("wv", [128, 8, 512])
    wo_d = din("wo", [128, 4, 1024])
    bq_d = din("bq", [128, 4], f32)
    bk_d = din("bk", [128, 4], f32)
    bvb_d = din("bvb", [128, 512], f32)
    mka_d = din("mka", [128, NT], f32)
    mkaL_d = din("mkaL", [128, HPC, NT], f32)
    mkaR_d = din("mkaR", [128, HPC, NT], f32)
    str_d = din("strips", [HPC, 2, cmax, 128, 1024], f8)
    out_d = nc.dram_tensor("outT", [D, S], f16, kind="ExternalOutput").ap()

    with tile.TileContext(nc) as tc:
        with (
            tc.tile_pool(name="const", bufs=1) as cpool,
            tc.tile_pool(name="qk", bufs=1) as qkpool,
            tc.tile_pool(name="vp", bufs=1) as vpool,
            tc.tile_pool(name="wo", bufs=1) as wopool,
            tc.tile_pool(name="ps", bufs=2, space="PSUM") as ps,
        ):
            ident8 = cpool.tile([128, 128], f8)
            make_identity(nc, ident8)
            identw = cpool.tile([128, 128], f16)
            make_identity(nc, identw)
            bq_s = cpool.tile([128, 4], f32)
            bk_s = cpool.tile([128, 4], f32)
            bvb_s = cpool.tile([128, 512], f32)
            mka_s = cpool.tile([128, NT], f32)
            mkaL_s = cpool.tile([128, HPC, NT], f32)
            mkaR_s = cpool.tile([128, HPC, NT], f32)
            nc.sync.dma_start(bq_s[:], bq_d[:])
            nc.sync.dma_start(bk_s[:], bk_d[:])
            nc.sync.dma_start(bvb_s[:], bvb_d[:])
            nc.sync.dma_start(mka_s[:], mka_d[:])
            nc.sync.dma_start(mkaL_s[:], mkaL_d[:])
            nc.sync.dma_start(mkaR_s[:], mkaR_d[:])

            qt = qkpool.tile([128, 4, 2048], f16, tag="qt")
            kt = qkpool.tile([128, 4, NKPAD], f16, tag="kt")
            vaug = vpool.tile([128, NT, 520], f16)
            wo_s = wopool.tile([128, 4, 1024], f16)
            # ones column of V_aug ([V|1]: denominator lands on partition 64)
            nc.vector.memset(
                vaug.rearrange("p k (h e) -> p k h e", h=8)[:, :, :, 64:65], 1.0)

            # warm up the PE clock (HAM) while input DMAs stream in
            jnk = cpool.tile([128, 512], f16)
            jnko = cpool.tile([128, 512], f16)
            nc.gpsimd.memset(jnk[:], 0.0)
            # pre-load the ACT exp table during the DMA window
            nc.scalar.activation(jnko[:], jnk[:], AF.Exp, scale=0.125)
            wmt = ps.tile([128, 1024], f32, tag="ps")
            for _ in range(56):
                nc.tensor.matmul(wmt[:, 0:512], identw[:], jnk[:],
                                 start=True, stop=True)

            # ---- phase 1: load x^T chunks + weights, projections ----
            with tc.tile_pool(name="xt", bufs=1) as xt_pool:
                wq_s = xt_pool.tile([128, 8, 512], f16, tag="wq")
                wk_s = xt_pool.tile([128, 8, 512], f16, tag="wk")
                wv_s = xt_pool.tile([128, 8, 512], f16, tag="wv")
                xq_c = [xt_pool.tile([128, 2048], f16, tag=f"xq{c}",
                                     name=f"xq{c}") for c in range(8)]
                xk_c = [xt_pool.tile([128, NKPAD], f16, tag=f"xk{c}",
                                     name=f"xk{c}") for c in range(8)]
                xv_c = [xt_pool.tile([128, NKPAD], f16, tag=f"xv{c}",
                                     name=f"xv{c}") for c in range(8)]
                # DMA order = consumption order: wq, xq chunks, wk, xk, wv, xv
                nc.sync.dma_start(wq_s[:], wq_d[:])
                for c in range(8):
                    nc.sync.dma_start(xq_c[c][:], xq_d[c])
                nc.sync.dma_start(wk_s[:], wk_d[:])
                for c in range(8):
                    nc.sync.dma_start(xk_c[c][:], xk_d[c])
                nc.sync.dma_start(wv_s[:], wv_d[:])
                for c in range(8):
                    nc.sync.dma_start(xv_c[c][:], xv_d[c])
                nc.sync.dma_start(wo_s[:], wo_d[:])

                # Q^T projection: out [pair-feat 128, seq 512]
                for p in range(NPAIR):
                    for s2 in range(2):
                        pt = ps.tile([128, 1024], f32, tag="ps")
                        for jq in range(2):
                            for c in range(8):
                                nc.tensor.matmul(
                                    pt[:, jq * 512:(jq + 1) * 512],
                                    wq_s[:, c, p * 128:(p + 1) * 128],
                                    xq_c[c][:, s2 * 1024 + jq * 512:
                                            s2 * 1024 + (jq + 1) * 512],
                                    start=(c == 0), stop=(c == 7))
                        nc.vector.tensor_scalar_add(
                            qt[:, p, s2 * 1024:(s2 + 1) * 1024], pt[:], bq_s[:, p:p + 1])
                # K^T projection over compacted+padded keys
                for p in range(NPAIR):
                    for (b0, bw) in plan['kblocks']:
                        pt = ps.tile([128, 1024], f32, tag="ps")
                        for (o0, ow) in ((0, min(512, bw)), (512, bw - 512)):
                            if ow <= 0:
                                continue
                            for c in range(8):
                                nc.tensor.matmul(
                                    pt[:, o0:o0 + ow],
                                    wk_s[:, c, p * 128:(p + 1) * 128],
                                    xk_c[c][:, b0 + o0:b0 + o0 + ow],
                                    start=(c == 0), stop=(c == 7))
                        nc.vector.tensor_scalar_add(
                            kt[:, p, b0:b0 + bw], pt[:, 0:bw], bk_s[:, p:p + 1])
                # V: out [keys 128, dv 512] per compacted key-tile
                for i in range(NT):
                    pt = ps.tile([128, 1024], f32, tag="ps")
                    acc = pt[:, 0:512]
                    for c in range(8):
                        nc.tensor.matmul(
                            acc, xv_c[c][:, i * 128:(i + 1) * 128],
                            wv_s[:, c, :], start=(c == 0), stop=(c == 7))
                    nc.vector.tensor_add(
                        vaug[:, i, :].rearrange("p (h e) -> p h e", h=8)[:, :, 0:64],
                        acc.rearrange("p (h e) -> p h e", e=64),
                        bvb_s.rearrange("p (h e) -> p h e", e=64))

            # ---- phase 2: attention over compacted keys ----
            with (
                tc.tile_pool(name="strips", bufs=2) as spool,
                tc.tile_pool(name="es", bufs=8) as espool,
                tc.tile_pool(name="ctxn", bufs=1) as cnpool,
                tc.tile_pool(name="rc", bufs=2) as rcpool,
                tc.tile_pool(name="cx", bufs=2, space="PSUM") as cx,
                tc.tile_pool(name="oev", bufs=4) as oevpool,
            ):
                ctxn_pj = [[cnpool.tile([128, 1024], f16, tag=f"ctxn{_p}_{_j}",
                                        name=f"ctxn{_p}_{_j}")
                            for _j in range(2)] for _p in range(NPAIR)]

                def phase3_half(jqq):
                    # output projection for query cols [jqq*1024, (jqq+1)*1024)
                    # per-pair ctxn tiles let c=0..2 fire before the last
                    # pair's normalize lands; tail copies go on idle ACT
                    for d in range(8):
                        pt = ps.tile([128, 1024], f32, tag="ps")
                        if d == 0:
                            # hold the HAM clock while the last normalize lands
                            for _ in range(10):
                                nc.tensor.matmul(pt[:, 0:512], identw[:],
                                                 jnk[:], start=True, stop=True)
                        for jq in range(2):
                            for c in range(4):
                                nc.tensor.matmul(
                                    pt[:, jq * 512:(jq + 1) * 512],
                                    wo_s[:, c, d * 128:(d + 1) * 128],
                                    ctxn_pj[c][jqq][:, jq * 512:(jq + 1) * 512],
                                    start=(c == 0), stop=(c == 3))
                        oev = oevpool.tile([128, 1024], f16, tag="oev")
                        if jqq == 1:
                            nc.scalar.copy(oev[:], pt[:])
                        else:
                            nc.vector.tensor_copy(oev[:], pt[:])
                        nc.sync.dma_start(
                            out_d[d * 128:(d + 1) * 128,
                                  jqq * 1024:(jqq + 1) * 1024],
                            oev[:])

                for jh in range(2):
                    for p in range(NPAIR):
                        strip = spool.tile([128, 2, cmax, 1024], f8,
                                           tag="strip", name="strip")
                        for e in range(2):
                            for n, i in enumerate(chunks[jh]):
                                nc.sync.dma_start(strip[:, e, n, :],
                                                  str_d[2 * p + e, jh, n])
                        cxt = [cx.tile([65, 1024], f32, tag="cx", name=f"cxt{_e}")
                               for _e in range(2)]
                        for i in range(NT):
                            for e in range(2):
                                h = 2 * p + e
                                side = skips.get((i, jh))
                                st = ps.tile([128, 1024], f32, tag="ps")
                                if i == 0 and e == 0:
                                    # keep the PE HAM clock warm through the
                                    # iteration-boundary bubble; QK's
                                    # start=True overwrite makes it harmless
                                    nfill = 10 if p == 0 else 2
                                    for _ in range(nfill):
                                        nc.tensor.matmul(
                                            st[:, 0:512], identw[:], jnk[:],
                                            start=True, stop=True)
                                for jq in range(2):
                                    nc.tensor.matmul(
                                        st[:, jq * 512:(jq + 1) * 512],
                                        kt[64 * e:64 * e + 64, p, i * 128:(i + 1) * 128],
                                        qt[64 * e:64 * e + 64, p,
                                           jh * 1024 + jq * 512:
                                           jh * 1024 + (jq + 1) * 512],
                                        start=True, stop=side is not None)
                                if side is None:
                                    n = ci[jh][i]
                                    for jq in range(2):
                                        nc.tensor.matmul(
                                            st[:, jq * 512:(jq + 1) * 512],
                                            ident8[:],
                                            strip[:, e, n, jq * 512:(jq + 1) * 512],
                                            start=False, stop=True)
                                    bias_ap = mka_s[:, i:i + 1]
                                else:
                                    bsel = mkaL_s if side == 'L' else mkaR_s
                                    bias_ap = bsel[:, h, i:i + 1]
                                es = espool.tile([128, 1024], f16, tag="es")
                                nc.scalar.activation(es[:], st[:], AF.Exp,
                                                     bias=bias_ap, scale=0.125)
                                for jq in range(2):
                                    nc.tensor.matmul(
                                        cxt[e][:, jq * 512:(jq + 1) * 512],
                                        vaug[:, i, 65 * h:65 * h + 65],
                                        es[:, jq * 512:(jq + 1) * 512],
                                        start=(i == 0), stop=(i == NT - 1))
                        # normalize both head-halves; recips emitted before the
                        # gpsimd-gated muls so the strict DVE FIFO pipelines
                        last = (jh == 1 and p == NPAIR - 1)
                        rcbs, stgs = [], []
                        for e in range(2):
                            den = rcpool.tile([1, 1024], f32, tag=f"den{e}")
                            rcp = rcpool.tile([1, 1024], f32, tag=f"rcp{e}")
                            rcb = rcpool.tile([64, 1024], f32, tag=f"rcb{e}")
                            nc.vector.tensor_copy(den[:], cxt[e][64:65, :])
                            if last:
                                stg = cxt[e][0:64, :]  # tail: mul from PSUM
                            else:
                                stg = rcpool.tile([64, 1024], f16, tag=f"stg{e}")
                                nc.vector.tensor_copy(stg[:], cxt[e][0:64, :])
                                stg = stg[:]
                            nc.vector.reciprocal_approx_fast(rcp[:], den[:])
                            nc.gpsimd.partition_broadcast(rcb[:], rcp[:])
                            rcbs.append(rcb)
                            stgs.append(stg)
                        for e in range(2):
                            nc.vector.tensor_mul(
                                ctxn_pj[p][jh][64 * e:64 * e + 64, :],
                                stgs[e], rcbs[e][:])
                        if jh == 1 and p == 0:
                            # jh0 output half: all jh0 ctxn ready; overlaps jh1
                            phase3_half(0)
                phase3_half(1)

    nc.compile()
    return nc


def _host_inputs(plan, query, key, value, mask, Wq, bq, Wk, bk, Wv, bv, Wo, bo,
                 rel_emb):
    f16 = np.float16
    f8 = ml_dtypes.float8_e4m3
    NT, NKPAD = plan['NT'], plan['NKPAD']
    chunks, cmax = plan['chunks'], plan['cmax']
    rel_np = np.asarray(rel_emb).astype(np.float32)
    in_maps = []
    for c in range(NCORES):
        b, g = divmod(c, 2)
        sl = slice(512 * g, 512 * (g + 1))
        hsl = slice(8 * g, 8 * (g + 1))
        ix = plan['idxs'][b]
        nk = len(ix)
        ko = np.zeros(NKPAD, np.int64)
        ko[:nk] = ix
        valid = np.zeros(NKPAD, bool)
        valid[:nk] = True
        # per-partition-per-tile padding bias (compacted keys are all unmasked)
        mka = np.where(valid.reshape(NT, 128).T, 0.0, MASK_NEG).astype(np.float32)
        mkaL = mka[:, None, :] + rel_np[0, hsl][None, :, None]    # [128,8,NT]
        mkaR = mka[:, None, :] + rel_np[256, hsl][None, :, None]  # [128,8,NT]
        # exact bias chunks [HPC, 2, cmax, 128, 1024]
        strips = np.zeros((HPC, 2, cmax, 128, 1024), np.float32)
        for jh in range(2):
            qs = jh * 1024 + np.arange(1024)[None, :]
            for n, i in enumerate(chunks[jh]):
                kv = ko[128 * i:128 * (i + 1), None]
                d = np.clip(kv - qs, -128, 128) + 128          # [128,1024]
                vals = 8.0 * rel_np[d][:, :, hsl]              # [128,1024,8]
                vals[~valid[128 * i:128 * (i + 1)], :, :] = 0.0
                strips[:, jh, n] = vals.transpose(2, 0, 1)

        def gather_pad(x):
            xg = np.zeros((NKPAD, D), np.float32)
            xg[:nk] = np.asarray(x)[ix]
            return xg

        def xt(xa):  # [N, D] -> [8, 128, N] transposed chunks
            return np.ascontiguousarray(xa.T.reshape(8, 128, -1)).astype(f16)

        def wcol(W):  # W.T[:, sl] [1024,512] -> [128, 8, 512]
            wt = np.asarray(W).T[:, sl].reshape(8, 128, 512)
            return np.ascontiguousarray(wt.transpose(1, 0, 2)).astype(f16)

        wo_t = np.asarray(Wo).T[sl, :].reshape(4, 128, 1024)
        in_maps.append({
            "xqt": xt(np.asarray(query[b]).astype(np.float32)),
            "xkt": xt(gather_pad(key[b])),
            "xvt": xt(gather_pad(value[b])),
            "wq": wcol(Wq),
            "wk": wcol(Wk),
            "wv": wcol(Wv),
            "wo": np.ascontiguousarray(wo_t.transpose(1, 0, 2)).astype(f16),
            "bq": np.ascontiguousarray(np.asarray(bq)[sl].reshape(4, 128).T).astype(np.float32),
            "bk": np.ascontiguousarray(np.asarray(bk)[sl].reshape(4, 128).T).astype(np.float32),
            "bvb": np.tile(np.asarray(bv)[sl].astype(np.float32), (128, 1)),
            "mka": np.ascontiguousarray(mka).astype(np.float32),
            "mkaL": np.ascontiguousarray(mkaL).astype(np.float32),
            "mkaR": np.ascontiguousarray(mkaR).astype(np.float32),
            "strips": strips.astype(f8),
        })
    return in_maps


def kernel(query, key, value, mask, Wq, bq, Wk, bk, Wv, bv, Wo, bo, rel_emb,
           _trace=False, _trace_kwargs=None):
    from concourse import bass_utils
    mkey = hashlib.sha1(np.ascontiguousarray(np.asarray(mask)).tobytes()).hexdigest()
    if _CACHE.get("mkey") != mkey:
        plan = _plan(mask)
        _CACHE.update(mkey=mkey, plan=plan, nc=_build(plan))
    nc, plan = _CACHE["nc"], _CACHE["plan"]
    in_maps = _host_inputs(plan, query, key, value, mask, Wq, bq, Wk, bk,
                           Wv, bv, Wo, bo, rel_emb)
    res = bass_utils.run_bass_kernel_spmd(
        nc, in_maps, core_ids=list(range(NCORES)), trace=_trace,
        **(_trace_kwargs or {}))
    _CACHE["last_res"] = res
    out = np.zeros((B, S, D), np.float32)
    for b in range(B):
        acc = (res.results[2 * b]["outT"].astype(np.float32)
               + res.results[2 * b + 1]["outT"].astype(np.float32))
        out[b] = acc.T
    out += np.asarray(bo).astype(np.float32)[None, None, :]
    return out
